# revision 50
# baseline (speedup 1.0000x reference)
"""Trainium2 Bass kernel for nn_Attention_52407190945839 (channel attention).

Single SPMD pass over 8 cores, data parallel over (batch, 64-row strips
of H).  Device computes the fused 9-tap qkv conv, the q/k Gram matrices
and the int8-quantized v; host sums Gram partials per batch, runs the
tiny 16x16-per-head attention math and applies
y = (proj @ blockdiag(attn)) @ v as one 128x128 sgemm per core.

The run is tunnel-bandwidth-bound (~45 MB/s to the axon-proxied cores,
~120 ms per-transfer latency), so the device invocation bypasses
run_bass_kernel_spmd's numpy path (which uploads host zeros for every
donated output buffer and re-uploads inputs synchronously inside the
measured call) and instead:
  - stages the int8-quantized inputs with one bulk sharded device_put
    issued asynchronously while the host still quantizes,
  - donates device-created zero buffers (jnp.zeros under jit -- no
    tunnel traffic),
  - packs all three logical outputs (v int8, per-head Gram blocks f32,
    v scales f16) into one int8 DRAM tensor per core via bitcast DMAs
    so the result comes back in a single bulk fetch,
  - issues the device call from the staging thread so execution overlaps
    the final host-side joins.

A 7-bit v encoding (12.5% smaller fetch) was built and validated
(kernel7.py) but rejected: it lands at 1.48e-2 fro / 1.59e-2 absmax
against the 2e-2 gate, versus 0.98e-2 / 1.12e-2 here -- not worth ~40ms.

The bass builder lives in BUILDER_SRC, exec'd under a fixed filename so
the instruction debug info (file + line, part of the BIR bytes and hence
the NEFF compilation cache key) is independent of this file's layout.
"""

import math
import os
import threading
import time

import numpy as np

DIM = 128
HEADS = 8
C = DIM // HEADS       # 16
H = W = 256
B = 2
N_CORES = 8
ROWS = H // 4          # 64 rows per core
L_CORE = ROWS * W      # 16384 positions per core
NTILE = L_CORE // 512  # 32 tiles of 512 for v quantization
PACK_V = L_CORE             # int8 v columns
PACK_G = 4 * 3 * C          # per-head qq|qk|kk gram blocks, f32 bytes
PACK_S = 2 * NTILE          # v-scale f16 bytes
PACK_COLS = PACK_V + PACK_G + PACK_S  # 16640

CACHE_DIR = "/root/.cache/bass_jax_cache"
BUILD_DIR = "/root/.cache/bass_kernel_build"
GROUPS = [[0, 1, 2, 3], [4, 5, 6, 7]]

LAST_TIMING = {}


# ------------------------------------------------------------- builder ----
# Exec'd under a fixed filename (BUILD_DIR/bass_builder_fixed.py) so BIR
# debug info -- and with it the NEFF cache key -- never depends on where
# kernel.py lives or how the host-side code below changes.

BUILDER_SRC = '''\
"""Fixed-path bass builder (generated from kernel.py BUILDER_SRC)."""
from contextlib import ExitStack

DIM = 128
HEADS = 8
C = 16
N_CORES = 8
ROWS = 64
W = 256
L_CORE = ROWS * W
NTILE = L_CORE // 512
NCHUNK = L_CORE // 128
PACK_V = L_CORE
PACK_G = 4 * 3 * C
PACK_S = 2 * NTILE
PACK_COLS = PACK_V + PACK_G + PACK_S


def build_kernels_into(holder, bass, bacc, mybir, tile):
    try:
        holder["ncs"] = (_build_kernel_a(bass, bacc, mybir, tile),)
    except BaseException as exc:  # noqa: BLE001
        holder["err"] = exc


def _build_kernel_a(bass, bacc, mybir, tile):
    """xpad, xscl, qkvt, dwt -> packed per-core output:
    [128, 16384 int8 v | 192 B gram blocks f32 | 64 B vscale f16]."""
    nc = bacc.Bacc("TRN2", target_bir_lowering=False, debug=False,
                   num_devices=N_CORES)
    f32 = mybir.dt.float32
    fp16 = mybir.dt.float16
    i8 = mybir.dt.int8
    xpad = nc.dram_tensor("xpad", [DIM, ROWS + 2, W + 2], i8,
                          kind="ExternalInput").ap()
    xscl = nc.dram_tensor("xscl", [DIM, ROWS + 2], f32,
                          kind="ExternalInput").ap()
    qkvt = nc.dram_tensor("qkvt", [DIM, 3 * DIM], f32,
                          kind="ExternalInput").ap()
    dwt_h = nc.dram_tensor("dwt", [9, 3 * DIM], f32, kind="ExternalInput")
    pout = nc.dram_tensor("pout", [DIM, PACK_COLS], i8,
                          kind="ExternalOutput")
    pout_v = pout.ap()
    pout_g = pout.ap()[:, PACK_V:PACK_V + PACK_G].bitcast(f32)
    pout_s = pout.ap()[:, PACK_V + PACK_G:PACK_COLS].bitcast(fp16)
    ALU = mybir.AluOpType

    with tile.TileContext(nc) as tc, ExitStack() as ctx:
        const = ctx.enter_context(tc.tile_pool(name="const", bufs=1))
        qkpool = ctx.enter_context(tc.tile_pool(name="qksb", bufs=4))
        vpool = ctx.enter_context(tc.tile_pool(name="vsb", bufs=3))
        gsb_pool = ctx.enter_context(tc.tile_pool(name="gsb", bufs=1))
        psqk = ctx.enter_context(tc.tile_pool(name="psqk", bufs=2, space="PSUM"))
        psg = ctx.enter_context(tc.tile_pool(name="psg", bufs=1, space="PSUM"))
        psv = ctx.enter_context(tc.tile_pool(name="psv", bufs=2, space="PSUM"))

        # expand the 9-tap conv weights on device:
        # w_tap[i, o] = qkv2T[i, o] * dwT[tap, o]; o 0..255 -> wqk, 256.. -> wv
        qkvt_sb = const.tile([DIM, 3 * DIM], f32)
        nc.sync.dma_start(qkvt_sb[:], qkvt)
        wqk_sb = const.tile([DIM, 9 * 2 * DIM], fp16)
        wv_sb = const.tile([DIM, 9 * DIM], fp16)
        for tap in range(9):
            dwb = const.tile([DIM, 3 * DIM], f32, tag="dwb", bufs=2)
            nc.sync.dma_start(
                dwb[:],
                bass.AP(tensor=dwt_h, offset=tap * 3 * DIM,
                        ap=[[0, DIM], [1, 3 * DIM]]))
            nc.vector.tensor_mul(wqk_sb[:, tap * 2 * DIM:(tap + 1) * 2 * DIM],
                                 qkvt_sb[:, :2 * DIM], dwb[:, :2 * DIM])
            nc.vector.tensor_mul(wv_sb[:, tap * DIM:(tap + 1) * DIM],
                                 qkvt_sb[:, 2 * DIM:], dwb[:, 2 * DIM:])
        xq_sb = const.tile([DIM, ROWS + 2, W + 2], i8)
        for lo, hi in [(0, 18), (18, 34), (34, 50), (50, ROWS + 2)]:
            nc.sync.dma_start(xq_sb[:, lo:hi, :], xpad[:, lo:hi, :])
        xs_sb = const.tile([DIM, ROWS + 2], f32)
        nc.sync.dma_start(xs_sb[:], xscl)
        xsb = const.tile([DIM, ROWS + 2, W + 2], fp16)
        for r in range(ROWS + 2):
            nc.vector.tensor_scalar(out=xsb[:, r, :], in0=xq_sb[:, r, :],
                                    scalar1=xs_sb[:, r:r + 1], scalar2=None,
                                    op0=ALU.mult)

        vs_sb = const.tile([DIM, NTILE], f32)  # per-(channel, tile) scales

        g1 = psg.tile([DIM, 2 * DIM], f32)   # q.q | q.k
        g2 = psg.tile([DIM, DIM], f32)       # k.k
        for rp in range(NTILE):
            # v tile: rows 2rp, 2rp+1 -> [128 ch, 512 pos], int8-quantized
            # against the per-channel abs-max of the tile
            pv = psv.tile([DIM, 512], f32)
            for tap in range(9):
                dh, dw = divmod(tap, 3)
                nc.tensor.matmul(
                    pv[:],
                    lhsT=wv_sb[:, tap * DIM:(tap + 1) * DIM],
                    rhs=xsb[:, 2 * rp + dh:2 * rp + dh + 2, dw:dw + W],
                    start=(tap == 0), stop=(tap == 8),
                )
            vm = vpool.tile([DIM, 1], f32)
            nc.vector.tensor_reduce(out=vm[:], in_=pv[:],
                                    axis=mybir.AxisListType.X, op=ALU.max,
                                    apply_absolute_value=True)
            nc.vector.tensor_scalar_max(vm[:], vm[:], 1e-30)
            vr = vpool.tile([DIM, 1], f32)
            nc.vector.reciprocal(vr[:], vm[:])
            vrs = vpool.tile([DIM, 1], f32)
            nc.vector.tensor_scalar_mul(vrs[:], vr[:], 127.0)
            nc.vector.tensor_scalar_mul(vs_sb[:, rp:rp + 1], vm[:], 1.0 / 127.0)
            # the DVE float->int8 cast rounds half-to-even (probed on HW)
            v_sb = vpool.tile([DIM, 512], i8)
            nc.vector.tensor_scalar(out=v_sb[:], in0=pv[:], scalar1=vrs[:],
                                    scalar2=None, op0=ALU.mult)
            nc.sync.dma_start(pout_v[:, rp * 512:(rp + 1) * 512], v_sb[:])

            # 4 qk/Gram chunks covering the same two rows
            for sub in range(4):
                ch = 4 * rp + sub
                r, wi = divmod(ch, 2)
                w0 = wi * 128
                pqk = psqk.tile([DIM, 2 * DIM], f32)
                for tap in range(9):
                    dh, dw = divmod(tap, 3)
                    nc.tensor.matmul(
                        pqk[:],
                        lhsT=xsb[:, r + dh, w0 + dw:w0 + dw + 128],
                        rhs=wqk_sb[:, tap * 2 * DIM:(tap + 1) * 2 * DIM],
                        start=(tap == 0), stop=(tap == 8),
                    )
                qkt = qkpool.tile([DIM, 2 * DIM], f32)
                nc.vector.tensor_copy(out=qkt[:], in_=pqk[:])
                nc.tensor.matmul(g1[:], lhsT=qkt[:, :DIM], rhs=qkt[:],
                                 start=(ch == 0), stop=(ch == NCHUNK - 1))
                nc.tensor.matmul(g2[:], lhsT=qkt[:, DIM:], rhs=qkt[:, DIM:],
                                 start=(ch == 0), stop=(ch == NCHUNK - 1))

        # only the per-head 16x16 blocks of the Gram matrices are used by
        # the host (diag of qq/kk + the qk block), so emit just those:
        # columns [0:16] qq block, [16:32] qk block, [32:48] kk block.
        # Compute engines can't address partition bases off the quadrant
        # grid, so the block extraction runs on the DMA engines instead.
        gsb = gsb_pool.tile([DIM, 3 * DIM], f32)
        nc.vector.tensor_copy(out=gsb[:, :2 * DIM], in_=g1[:])
        nc.vector.tensor_copy(out=gsb[:, 2 * DIM:], in_=g2[:])
        for h in range(HEADS):
            p0 = h * C
            nc.sync.dma_start(pout_g[p0:p0 + C, 0:C],
                              gsb[p0:p0 + C, p0:p0 + C])
            nc.sync.dma_start(pout_g[p0:p0 + C, C:2 * C],
                              gsb[p0:p0 + C, DIM + p0:DIM + p0 + C])
            nc.sync.dma_start(pout_g[p0:p0 + C, 2 * C:3 * C],
                              gsb[p0:p0 + C, 2 * DIM + p0:2 * DIM + p0 + C])
        vs16 = gsb_pool.tile([DIM, NTILE], fp16)
        nc.vector.tensor_copy(out=vs16[:], in_=vs_sb[:])
        nc.sync.dma_start(pout_s, vs16[:])
    nc.compile()
    return nc
'''


def _builder_module():
    """Exec BUILDER_SRC under a fixed filename and return the module.

    The file is also written out (best-effort) for inspectability, but the
    code objects always carry the fixed path + BUILDER_SRC line numbers, so
    the BIR debug info -- and the NEFF cache key -- is stable no matter
    where kernel.py lives or how its host code changes.
    """
    import types

    fixed = os.path.join(BUILD_DIR, "bass_builder_fixed.py")
    try:
        os.makedirs(BUILD_DIR, exist_ok=True)
        old = ""
        if os.path.exists(fixed):
            with open(fixed) as f:
                old = f.read()
        if old != BUILDER_SRC:
            with open(fixed, "w") as f:
                f.write(BUILDER_SRC)
    except Exception:
        pass
    mod = types.ModuleType("bass_builder_fixed")
    code = compile(BUILDER_SRC, fixed, "exec")
    exec(code, mod.__dict__)  # noqa: S102
    return mod


# ---------------------------------------------------------------- host ----

def _quantize(x):
    """int8-quantize x with per-(batch, channel, global-row) scales and
    slice into the 8 per-core padded strips (one halo row each side)."""
    m = np.maximum(np.abs(x).max(axis=3), 1e-30)           # [B, DIM, H]
    xq = np.rint(x * (127.0 / m)[..., None]).astype(np.int8)
    scl = (m * (1.0 / 127.0)).astype(np.float32)
    xpad_all = np.zeros((N_CORES * DIM, ROWS + 2, W + 2), np.int8)
    xscl_all = np.zeros((N_CORES * DIM, ROWS + 2), np.float32)
    xp4 = xpad_all.reshape(N_CORES, DIM, ROWS + 2, W + 2)
    xs3 = xscl_all.reshape(N_CORES, DIM, ROWS + 2)
    for core in range(N_CORES):
        b, quad = divmod(core, 4)
        r0 = quad * ROWS
        j0 = 1 if r0 == 0 else 0
        j1 = ROWS + 1 if r0 + ROWS == H else ROWS + 2
        g0 = r0 - 1 + j0
        xp4[core, :, j0:j1, 1:W + 1] = xq[b, :, g0:g0 + (j1 - j0), :]
        xs3[core, :, j0:j1] = scl[b, :, g0:g0 + (j1 - j0)]
    return xpad_all, xscl_all


def _attention_host(grams, proj_w, attca_w, temperature):
    """Per-batch: per-head gram blocks [128, qq|qk|kk] -> attention math
    -> M = proj @ blockdiag(attn)."""
    f = np.float32
    attca2 = attca_w[:, :, 0, 0].astype(f)   # [32, 16]
    projT = proj_w[:, :, 0, 0].T.astype(f)   # [i, o]
    temp128 = np.repeat(temperature.reshape(HEADS).astype(f), C)[:, None]
    ridx = np.arange(DIM)

    ms = []
    for b in range(B):
        g = np.sum([grams[c] for c in GROUPS[b]], axis=0, dtype=np.float64)
        g = g.astype(np.float64)
        sq2 = g[ridx, ridx % C]              # qq block diagonals
        sk2 = g[ridx, 2 * C + ridx % C]      # kk block diagonals
        rq = 1.0 / np.maximum(np.sqrt(sq2), 1e-12)
        rk = 1.0 / np.maximum(np.sqrt(sk2), 1e-12)
        attnb0 = g[:, C:2 * C]               # qk per-head blocks
        rkb = rk.reshape(HEADS, C)[ridx // C, :]
        attn = attnb0 * rkb * rq[:, None] * temp128
        m = attn.max(axis=1, keepdims=True)
        e = np.exp(attn - m)
        attn0 = e / e.sum(axis=1, keepdims=True)
        rl = np.maximum(attn, 0.0)
        r2 = rl * rl
        erf = np.vectorize(math.erf)
        gl = 0.5 * r2 * (1.0 + erf(r2 / math.sqrt(2.0)))
        a1 = gl * r2
        ss = a1 @ attca2.T  # [128, 32]
        attnf = attn0 * (1.0 + ss[:, :C]) + ss[:, C:]
        bd = np.zeros((DIM, DIM))
        for h in range(HEADS):
            bd[h * C:(h + 1) * C, h * C:(h + 1) * C] = attnf[h * C:(h + 1) * C]
        ms.append(np.ascontiguousarray((bd.T @ projT).T.astype(f)))  # M [o, d]
    return ms


# ---------------------------------------------------------------- device ----

def _make_runner(nc, n_cores):
    """Jitted shard_map over 8 cores around the prebuilt bass module --
    the same _bass_exec path run_bass_kernel_spmd uses under axon, minus
    its host-side concat / zero-upload / bulk-result round trips."""
    import jax
    import concourse.bass2jax as bass2jax
    import concourse.mybir as mybir
    from jax.sharding import Mesh, NamedSharding, PartitionSpec
    from jax.experimental.shard_map import shard_map

    bass2jax.install_neuronx_cc_hook()
    assert nc.dbg_addr is None, "debug builds not supported by this runner"
    partition_name = (nc.partition_id_tensor.name
                      if nc.partition_id_tensor else None)

    in_names, out_names, out_avals = [], [], []
    for alloc in nc.m.functions[0].allocations:
        if not isinstance(alloc, mybir.MemoryLocationSet):
            continue
        name = alloc.memorylocations[0].name
        if alloc.kind == "ExternalInput":
            if name != partition_name:
                in_names.append(name)
        elif alloc.kind == "ExternalOutput":
            out_names.append(name)
            out_avals.append(jax.core.ShapedArray(
                tuple(alloc.tensor_shape), mybir.dt.np(alloc.dtype)))
    assert in_names == ["xpad", "xscl", "qkvt", "dwt"], in_names
    assert out_names == ["pout"], out_names
    n_params, n_outs = len(in_names), len(out_names)
    all_names = in_names + out_names
    if partition_name is not None:
        all_names = all_names + [partition_name]

    def _body(*args):
        operands = list(args)
        if partition_name is not None:
            operands.append(bass2jax.partition_id_tensor())
        outs = bass2jax._bass_exec_p.bind(
            *operands,
            out_avals=tuple(out_avals),
            in_names=tuple(all_names),
            out_names=tuple(out_names),
            lowering_input_output_aliases=(),
            sim_require_finite=True,
            sim_require_nnan=True,
            nc=nc,
        )
        return tuple(outs)

    devices = jax.devices()[:n_cores]
    mesh = Mesh(np.asarray(devices), ("core",))
    in_specs = (PartitionSpec("core"),) * (n_params + n_outs)
    out_specs = (PartitionSpec("core"),) * n_outs
    donate = tuple(range(n_params, n_params + n_outs))
    sharded = jax.jit(
        shard_map(_body, mesh=mesh, in_specs=in_specs, out_specs=out_specs,
                  check_rep=False),
        donate_argnums=donate, keep_unused=True)
    sh = NamedSharding(mesh, PartitionSpec("core"))
    return sharded, sh


def _warmup_into(state):
    """Backend connect + bass build + jit/NEFF compile + executable load,
    run concurrently with host prep.  Ends with device-side zero buffers
    staged for the real call -- no tunnel traffic for any of it."""
    try:
        import jax

        try:
            os.makedirs(CACHE_DIR, exist_ok=True)
            os.environ.setdefault("JAX_COMPILATION_CACHE_DIR", CACHE_DIR)
            jax.config.update("jax_compilation_cache_dir", CACHE_DIR)
            jax.config.update("jax_persistent_cache_min_entry_size_bytes", -1)
            jax.config.update("jax_persistent_cache_min_compile_time_secs", 0.0)
        except Exception:
            pass  # cache is an optimization; run without it if unavailable

        import concourse.bass as bass
        import concourse.bacc as bacc
        import concourse.mybir as mybir
        import concourse.tile as tile
        import jax.numpy as jnp

        bmod = _builder_module()
        holder = {}
        th = threading.Thread(target=bmod.build_kernels_into,
                              name="bass-build",
                              args=(holder, bass, bacc, mybir, tile))
        th.start()
        # Connect + first-touch every device while the bass build runs, so
        # per-process transfer-path warmup stalls land here.
        devs = jax.devices()[:N_CORES]
        futs = [jax.device_put(np.ones((8, 8), np.float32), d) for d in devs]
        for fut in futs:
            fut.block_until_ready()
        state["dev_ready"].set()
        th.join()
        if "err" in holder:
            raise holder["err"]
        state["ncs"] = holder["ncs"]
        (nc_a,) = holder["ncs"]
        sharded, sh = _make_runner(nc_a, N_CORES)
        state["sharded"] = sharded

        # Device-side zeros: dummy inputs for the warmup execution (which
        # triggers the jit/NEFF compile or cache hit plus executable load on
        # all 8 cores) and a fresh donated output buffer for the real call.
        zin = jax.jit(
            lambda: (jnp.zeros((N_CORES * DIM, ROWS + 2, W + 2), jnp.int8),
                     jnp.zeros((N_CORES * DIM, ROWS + 2), jnp.float32),
                     jnp.zeros((N_CORES * DIM, 3 * DIM), jnp.float32),
                     jnp.zeros((N_CORES * 9, 3 * DIM), jnp.float32),
                     jnp.zeros((N_CORES * DIM, PACK_COLS), jnp.int8)),
            out_shardings=(sh, sh, sh, sh, sh))
        dummies = zin()
        zreal = zin()[4]
        douts = sharded(*dummies)
        jax.block_until_ready(douts)
        state["zeros"] = zreal
        # Warm the downstream path as the LAST warmup step: the first big
        # device->host transfer of a process runs ~80 ms slower than steady
        # state, and the TCP window decays again after idle gaps -- so the
        # warm fetch must end directly before the real call.  (Issuing the
        # real call before/during this fetch was tried and regressed: its
        # completion signal queues behind the 17 MB warm stream on the
        # single relay connection.)
        np.asarray(douts[0])
    except BaseException as exc:  # noqa: BLE001
        state["err"] = exc
    finally:
        state["dev_ready"].set()
        state["runner_ready"].set()
        state["warm_done"].set()


def _upload_into(state, qkv2T, dwT):
    """Stage inputs to the cores (small replicated weights as soon as the
    backend is up, the bulk int8 x as soon as quantization finishes), then
    issue the device call so execution overlaps the remaining host work."""
    try:
        state["dev_ready"].wait()
        if "err" in state:
            return
        import jax
        from jax.sharding import Mesh, NamedSharding, PartitionSpec

        devs = jax.devices()[:N_CORES]
        sh = NamedSharding(Mesh(np.asarray(devs), ("core",)),
                           PartitionSpec("core"))
        state["qkvt_arr"] = jax.device_put(
            np.tile(np.ascontiguousarray(qkv2T), (N_CORES, 1)), sh)
        state["dwt_arr"] = jax.device_put(
            np.tile(np.ascontiguousarray(dwT), (N_CORES, 1)), sh)
        state["quant_ready"].wait()
        state["x_arr"] = jax.device_put(state["xpad_all"], sh)
        state["xs_arr"] = jax.device_put(state["xscl_all"], sh)
        state["warm_done"].wait()
        if "err" in state:
            return
        state["outs"] = state["sharded"](state["x_arr"], state["xs_arr"],
                                         state["qkvt_arr"], state["dwt_arr"],
                                         state["zeros"])
        # Register the device->host copy now so the transfer starts the
        # moment execution completes, overlapping the exec round trip.
        state["outs"][0].copy_to_host_async()
    except BaseException as exc:  # noqa: BLE001
        state["err_upload"] = exc


def _run_fallback(state, xpad_all, xscl_all, qkv2T, dwT):
    """Classic run_bass_kernel_spmd invocation (slower transfers, same
    kernel) -- used only if the custom runner path fails."""
    from concourse import bass_utils

    (nc_a,) = state["ncs"]
    xp4 = xpad_all.reshape(N_CORES, DIM, ROWS + 2, W + 2)
    xs3 = xscl_all.reshape(N_CORES, DIM, ROWS + 2)
    in_maps = [{"xpad": xp4[c], "xscl": xs3[c], "qkvt": qkv2T, "dwt": dwT}
               for c in range(N_CORES)]
    res = bass_utils.run_bass_kernel_spmd(nc_a, in_maps, list(range(N_CORES)))
    return np.stack([r["pout"] for r in res.results])


# ------------------------------------------------------------- emulation ----

def _emulate_device(xpad_all, xscl_all, qkv2T, dwT):
    """Numpy re-implementation of the device kernel: packed outputs."""
    f = np.float32
    wqk_f = np.empty((DIM, 9 * 2 * DIM), dtype=f)
    wv_f = np.empty((DIM, 9 * DIM), dtype=f)
    for tap in range(9):
        prod = qkv2T * dwT[tap][None, :]
        wqk_f[:, tap * 2 * DIM:(tap + 1) * 2 * DIM] = \
            prod[:, :2 * DIM].astype(np.float16).astype(f)
        wv_f[:, tap * DIM:(tap + 1) * DIM] = \
            prod[:, 2 * DIM:].astype(np.float16).astype(f)
    xp4 = xpad_all.reshape(N_CORES, DIM, ROWS + 2, W + 2)
    xs3 = xscl_all.reshape(N_CORES, DIM, ROWS + 2)
    packed = np.zeros((N_CORES, DIM, PACK_COLS), np.int8)
    for core in range(N_CORES):
        xp = (xp4[core].astype(f) * xs3[core][:, :, None]
              ).astype(np.float16).astype(f)
        qk = np.zeros((2 * DIM, ROWS, W), dtype=f)
        v = np.zeros((DIM, ROWS, W), dtype=f)
        for tap in range(9):
            dh, dw = divmod(tap, 3)
            xs = xp[:, dh:dh + ROWS, dw:dw + W]
            qk += np.einsum('io,ihw->ohw',
                            wqk_f[:, tap * 2 * DIM:(tap + 1) * 2 * DIM], xs)
            v += np.einsum('io,ihw->ohw',
                           wv_f[:, tap * DIM:(tap + 1) * DIM], xs)
        q = qk[:DIM].reshape(DIM, L_CORE)
        k = qk[DIM:].reshape(DIM, L_CORE)
        g = np.empty((DIM, 3 * DIM), dtype=f)
        g[:, :DIM] = q @ q.T
        g[:, DIM:2 * DIM] = q @ k.T
        g[:, 2 * DIM:] = k @ k.T
        vt = v.reshape(DIM, NTILE, 512)
        m = np.maximum(np.abs(vt).max(axis=2, keepdims=True), 1e-30)
        vq = np.rint(vt * (127.0 / m)).astype(np.int8)
        packed[core, :, :PACK_V] = vq.reshape(DIM, L_CORE)
        gc = np.empty((DIM, 3 * C), dtype=f)
        for h in range(HEADS):
            p0 = h * C
            gc[p0:p0 + C, 0:C] = g[p0:p0 + C, p0:p0 + C]
            gc[p0:p0 + C, C:2 * C] = g[p0:p0 + C, DIM + p0:DIM + p0 + C]
            gc[p0:p0 + C, 2 * C:] = g[p0:p0 + C, 2 * DIM + p0:2 * DIM + p0 + C]
        packed[core, :, PACK_V:PACK_V + PACK_G] = gc.view(np.int8)
        packed[core, :, PACK_V + PACK_G:] = \
            np.ascontiguousarray((m[:, :, 0] / 127.0).astype(np.float16)
                                 ).view(np.int8)
    return packed


# ---------------------------------------------------------------- entry ----

def kernel(x, qkv_w, dw_w, proj_w, attca_w, temperature):
    t_start = time.perf_counter()
    x = np.ascontiguousarray(np.asarray(x, dtype=np.float32))
    qkv_w = np.asarray(qkv_w, dtype=np.float32)
    dw_w = np.asarray(dw_w, dtype=np.float32)
    proj_w = np.asarray(proj_w, dtype=np.float32)
    attca_w = np.asarray(attca_w, dtype=np.float32)
    temperature = np.asarray(temperature, dtype=np.float32)

    qkv2T = np.ascontiguousarray(qkv_w[:, :, 0, 0].T)            # [128, 384]
    dwT = np.ascontiguousarray(dw_w[:, 0].reshape(3 * DIM, 9).T)  # [9, 384]

    emulate = os.environ.get("KERNEL_EMULATE", "0") == "1"
    state = {
        "dev_ready": threading.Event(),
        "quant_ready": threading.Event(),
        "runner_ready": threading.Event(),
        "warm_done": threading.Event(),
    }
    if not emulate:
        wth = threading.Thread(target=_warmup_into, args=(state,), daemon=True)
        uth = threading.Thread(target=_upload_into, args=(state, qkv2T, dwT),
                               daemon=True)
        wth.start()
        uth.start()

    xpad_all, xscl_all = _quantize(x)
    state["xpad_all"] = xpad_all
    state["xscl_all"] = xscl_all
    state["quant_ready"].set()

    if emulate:
        packed = _emulate_device(xpad_all, xscl_all, qkv2T, dwT)
        t1 = t2 = time.perf_counter()
    else:
        wth.join()
        uth.join()
        t1 = time.perf_counter()
        try:
            if "err" in state:
                raise state["err"]
            if "err_upload" in state:
                raise state["err_upload"]
            packed = np.asarray(state["outs"][0]).reshape(
                N_CORES, DIM, PACK_COLS)
        except Exception:
            try:
                if "ncs" not in state:
                    raise
                packed = _run_fallback(state, xpad_all, xscl_all, qkv2T, dwT)
            except Exception:
                packed = _emulate_device(xpad_all, xscl_all, qkv2T, dwT)
        t2 = time.perf_counter()

    grams = [np.ascontiguousarray(packed[c, :, PACK_V:PACK_V + PACK_G]
                                  ).view(np.float32)
             for c in range(N_CORES)]
    ms = _attention_host(grams, proj_w, attca_w, temperature)
    out = np.empty((B, DIM, H, W), dtype=np.float32)
    for core in range(N_CORES):
        b, quad = divmod(core, 4)
        r0 = quad * ROWS
        vs = np.ascontiguousarray(packed[core, :, PACK_V + PACK_G:]
                                  ).view(np.float16).astype(np.float32)
        vq = packed[core, :, :PACK_V].astype(np.float32)
        v = (vq.reshape(DIM, NTILE, 512) * vs[:, :, None]).reshape(DIM, L_CORE)
        out[b, :, r0:r0 + ROWS, :] = (ms[core // 4] @ v).reshape(DIM, ROWS, W)
    t3 = time.perf_counter()

    LAST_TIMING["build_a_s"] = t1 - t_start
    LAST_TIMING["run_a_s"] = t2 - t1
    LAST_TIMING["host_s"] = t3 - t2
    LAST_TIMING["run_b_s"] = 0.0
    LAST_TIMING["kernel_a_ns"] = None
    LAST_TIMING["kernel_b_ns"] = None
    return out


# Preload heavyweight modules at import time so kernel() only pays for the
# device connect, compiles and transfers.
try:
    import jax  # noqa: F401,E402
    import concourse.bass  # noqa: F401,E402
    import concourse.bacc  # noqa: F401,E402
    import concourse.mybir  # noqa: F401,E402
    import concourse.tile  # noqa: F401,E402
    import concourse.bass2jax  # noqa: F401,E402
    from concourse import bass_utils  # noqa: F401,E402
except Exception:  # pragma: no cover - emulation-only environments
    pass


# revision 51
# speedup vs baseline: 1.1152x; 1.1152x over previous
"""Trainium2 Bass kernel for nn_Attention_52407190945839 (channel attention).

Single SPMD pass over 8 cores, data parallel over (batch, 64-row strips
of H).  Device computes the fused 9-tap qkv conv, the q/k Gram matrices
and the int8-quantized v; host sums Gram partials per batch, runs the
tiny 16x16-per-head attention math and applies
y = (proj @ blockdiag(attn)) @ v as one 128x128 sgemm per core.

The run is tunnel-bandwidth-bound (~45 MB/s to the axon-proxied cores,
~120 ms per-transfer latency), so the device invocation bypasses
run_bass_kernel_spmd's numpy path (which uploads host zeros for every
donated output buffer and re-uploads inputs synchronously inside the
measured call) and instead:
  - stages the int8-quantized inputs with one bulk sharded device_put
    issued asynchronously while the host still quantizes,
  - donates device-created zero buffers (jnp.zeros under jit -- no
    tunnel traffic),
  - packs all three logical outputs (v int8, Gram f32, v scales f32)
    into one int8 DRAM tensor per core via bitcast DMAs so the result
    comes back in a single bulk fetch.

The bass builder lives in BUILDER_SRC, exec'd under a fixed filename so
the instruction debug info (file + line, part of the BIR bytes and hence
the NEFF compilation cache key) is independent of this file's layout.
"""

import math
import os
import threading
import time

import numpy as np

DIM = 128
HEADS = 8
C = DIM // HEADS       # 16
H = W = 256
B = 2
N_CORES = 8
ROWS = H // 4          # 64 rows per core
L_CORE = ROWS * W      # 16384 positions per core
NTILE = L_CORE // 512  # 32 tiles of 512 for v quantization
PACK_V = (L_CORE // 8) * 7  # 7-bit-packed v bytes (8 values -> 7 bytes)
PACK_G = 4 * 3 * C          # per-head qq|qk|kk gram blocks, f32 bytes
PACK_S = 2 * NTILE          # v-scale f16 bytes
PACK_COLS = PACK_V + PACK_G + PACK_S  # 14592

CACHE_DIR = "/root/.cache/bass_jax_cache"
BUILD_DIR = "/root/.cache/bass_kernel_build"
GROUPS = [[0, 1, 2, 3], [4, 5, 6, 7]]

LAST_TIMING = {}


# ------------------------------------------------------------- builder ----
# Exec'd under a fixed filename (BUILD_DIR/bass_builder_fixed.py) so BIR
# debug info -- and with it the NEFF cache key -- never depends on where
# kernel.py lives or how the host-side code below changes.

BUILDER_SRC = '''\
"""Fixed-path bass builder (generated from kernel.py BUILDER_SRC)."""
from contextlib import ExitStack

DIM = 128
HEADS = 8
C = 16
N_CORES = 8
ROWS = 64
W = 256
L_CORE = ROWS * W
NTILE = L_CORE // 512
NCHUNK = L_CORE // 128
PACK_V = (L_CORE // 8) * 7
PACK_G = 4 * 3 * C
PACK_S = 2 * NTILE
PACK_COLS = PACK_V + PACK_G + PACK_S


def build_kernels_into(holder, bass, bacc, mybir, tile):
    try:
        holder["ncs"] = (_build_kernel_a(bass, bacc, mybir, tile),)
    except BaseException as exc:  # noqa: BLE001
        holder["err"] = exc


def _build_kernel_a(bass, bacc, mybir, tile):
    """xpad (fp16), qkvt, dwt -> packed per-core output:
    [128, 14336 B 7-bit v | 192 B gram blocks f32 | 64 B vscale f16]."""
    nc = bacc.Bacc("TRN2", target_bir_lowering=False, debug=False,
                   num_devices=N_CORES)
    f32 = mybir.dt.float32
    fp16 = mybir.dt.float16
    u8 = mybir.dt.uint8
    xpad = nc.dram_tensor("xpad", [DIM, ROWS + 2, W + 2], fp16,
                          kind="ExternalInput").ap()
    qkvt = nc.dram_tensor("qkvt", [DIM, 3 * DIM], f32,
                          kind="ExternalInput").ap()
    dwt_h = nc.dram_tensor("dwt", [9, 3 * DIM], f32, kind="ExternalInput")
    pout = nc.dram_tensor("pout", [DIM, PACK_COLS], u8,
                          kind="ExternalOutput")
    pout_v = pout.ap()
    pout_g = pout.ap()[:, PACK_V:PACK_V + PACK_G].bitcast(f32)
    pout_s = pout.ap()[:, PACK_V + PACK_G:PACK_COLS].bitcast(fp16)
    ALU = mybir.AluOpType

    with tile.TileContext(nc) as tc, ExitStack() as ctx:
        const = ctx.enter_context(tc.tile_pool(name="const", bufs=1))
        qkpool = ctx.enter_context(tc.tile_pool(name="qksb", bufs=4))
        vpool = ctx.enter_context(tc.tile_pool(name="vsb", bufs=3))
        gsb_pool = ctx.enter_context(tc.tile_pool(name="gsb", bufs=1))
        psqk = ctx.enter_context(tc.tile_pool(name="psqk", bufs=2, space="PSUM"))
        psg = ctx.enter_context(tc.tile_pool(name="psg", bufs=1, space="PSUM"))
        psv = ctx.enter_context(tc.tile_pool(name="psv", bufs=2, space="PSUM"))

        # expand the 9-tap conv weights on device:
        # w_tap[i, o] = qkv2T[i, o] * dwT[tap, o]; o 0..255 -> wqk, 256.. -> wv
        qkvt_sb = const.tile([DIM, 3 * DIM], f32)
        nc.sync.dma_start(qkvt_sb[:], qkvt)
        wqk_sb = const.tile([DIM, 9 * 2 * DIM], fp16)
        wv_sb = const.tile([DIM, 9 * DIM], fp16)
        for tap in range(9):
            dwb = const.tile([DIM, 3 * DIM], f32, tag="dwb", bufs=2)
            nc.sync.dma_start(
                dwb[:],
                bass.AP(tensor=dwt_h, offset=tap * 3 * DIM,
                        ap=[[0, DIM], [1, 3 * DIM]]))
            nc.vector.tensor_mul(wqk_sb[:, tap * 2 * DIM:(tap + 1) * 2 * DIM],
                                 qkvt_sb[:, :2 * DIM], dwb[:, :2 * DIM])
            nc.vector.tensor_mul(wv_sb[:, tap * DIM:(tap + 1) * DIM],
                                 qkvt_sb[:, 2 * DIM:], dwb[:, 2 * DIM:])
        xsb = const.tile([DIM, ROWS + 2, W + 2], fp16)
        for lo, hi in [(0, 18), (18, 34), (34, 50), (50, ROWS + 2)]:
            nc.sync.dma_start(xsb[:, lo:hi, :], xpad[:, lo:hi, :])

        vs_sb = const.tile([DIM, NTILE], f32)  # per-(channel, tile) scales

        g1 = psg.tile([DIM, 2 * DIM], f32)   # q.q | q.k
        g2 = psg.tile([DIM, DIM], f32)       # k.k
        for rp in range(NTILE):
            # v tile: rows 2rp, 2rp+1 -> [128 ch, 512 pos], int8-quantized
            # against the per-channel abs-max of the tile
            pv = psv.tile([DIM, 512], f32)
            for tap in range(9):
                dh, dw = divmod(tap, 3)
                nc.tensor.matmul(
                    pv[:],
                    lhsT=wv_sb[:, tap * DIM:(tap + 1) * DIM],
                    rhs=xsb[:, 2 * rp + dh:2 * rp + dh + 2, dw:dw + W],
                    start=(tap == 0), stop=(tap == 8),
                )
            vm = vpool.tile([DIM, 1], f32)
            nc.vector.tensor_reduce(out=vm[:], in_=pv[:],
                                    axis=mybir.AxisListType.X, op=ALU.max,
                                    apply_absolute_value=True)
            nc.vector.tensor_scalar_max(vm[:], vm[:], 1e-30)
            vr = vpool.tile([DIM, 1], f32)
            nc.vector.reciprocal(vr[:], vm[:])
            vrs = vpool.tile([DIM, 1], f32)
            nc.vector.tensor_scalar_mul(vrs[:], vr[:], 63.0)
            nc.vector.tensor_scalar_mul(vs_sb[:, rp:rp + 1], vm[:], 1.0 / 63.0)
            # 7-bit quantize: u = round(pv * 63/max) + 64 in [1, 127]
            # (the DVE float->int cast rounds half-to-even, probed on HW)
            u8t = vpool.tile([DIM, 512], u8)
            nc.vector.tensor_scalar(out=u8t[:], in0=pv[:], scalar1=vrs[:],
                                    scalar2=64.0, op0=ALU.mult, op1=ALU.add)
            # pack 8 7-bit values into 7 bytes: byte i of each group gets
            # value i's low 7 bits plus bit i of value 7 in its MSB.
            vpk = vpool.tile([DIM, 448], u8)
            for i in range(7):
                tb = vpool.tile([DIM, 64], u8, tag="tb", bufs=2)
                nc.vector.tensor_scalar(out=tb[:], in0=u8t[:, 7::8],
                                        scalar1=i, scalar2=1,
                                        op0=ALU.logical_shift_right,
                                        op1=ALU.bitwise_and)
                tb7 = vpool.tile([DIM, 64], u8, tag="tb7", bufs=2)
                nc.vector.tensor_scalar(out=tb7[:], in0=tb[:],
                                        scalar1=7, scalar2=None,
                                        op0=ALU.logical_shift_left)
                nc.vector.tensor_tensor(out=vpk[:, i::7], in0=tb7[:],
                                        in1=u8t[:, i::8], op=ALU.bitwise_or)
            nc.sync.dma_start(pout_v[:, rp * 448:(rp + 1) * 448], vpk[:])

            # 4 qk/Gram chunks covering the same two rows
            for sub in range(4):
                ch = 4 * rp + sub
                r, wi = divmod(ch, 2)
                w0 = wi * 128
                pqk = psqk.tile([DIM, 2 * DIM], f32)
                for tap in range(9):
                    dh, dw = divmod(tap, 3)
                    nc.tensor.matmul(
                        pqk[:],
                        lhsT=xsb[:, r + dh, w0 + dw:w0 + dw + 128],
                        rhs=wqk_sb[:, tap * 2 * DIM:(tap + 1) * 2 * DIM],
                        start=(tap == 0), stop=(tap == 8),
                    )
                qkt = qkpool.tile([DIM, 2 * DIM], f32)
                nc.vector.tensor_copy(out=qkt[:], in_=pqk[:])
                nc.tensor.matmul(g1[:], lhsT=qkt[:, :DIM], rhs=qkt[:],
                                 start=(ch == 0), stop=(ch == NCHUNK - 1))
                nc.tensor.matmul(g2[:], lhsT=qkt[:, DIM:], rhs=qkt[:, DIM:],
                                 start=(ch == 0), stop=(ch == NCHUNK - 1))

        # only the per-head 16x16 blocks of the Gram matrices are used by
        # the host (diag of qq/kk + the qk block), so emit just those:
        # columns [0:16] qq block, [16:32] qk block, [32:48] kk block.
        # Compute engines can't address partition bases off the quadrant
        # grid, so the block extraction runs on the DMA engines instead.
        gsb = gsb_pool.tile([DIM, 3 * DIM], f32)
        nc.vector.tensor_copy(out=gsb[:, :2 * DIM], in_=g1[:])
        nc.vector.tensor_copy(out=gsb[:, 2 * DIM:], in_=g2[:])
        for h in range(HEADS):
            p0 = h * C
            nc.sync.dma_start(pout_g[p0:p0 + C, 0:C],
                              gsb[p0:p0 + C, p0:p0 + C])
            nc.sync.dma_start(pout_g[p0:p0 + C, C:2 * C],
                              gsb[p0:p0 + C, DIM + p0:DIM + p0 + C])
            nc.sync.dma_start(pout_g[p0:p0 + C, 2 * C:3 * C],
                              gsb[p0:p0 + C, 2 * DIM + p0:2 * DIM + p0 + C])
        vs16 = gsb_pool.tile([DIM, NTILE], fp16)
        nc.vector.tensor_copy(out=vs16[:], in_=vs_sb[:])
        nc.sync.dma_start(pout_s, vs16[:])
    nc.compile()
    return nc
'''


def _builder_module():
    """Exec BUILDER_SRC under a fixed filename and return the module.

    The file is also written out (best-effort) for inspectability, but the
    code objects always carry the fixed path + BUILDER_SRC line numbers, so
    the BIR debug info -- and the NEFF cache key -- is stable no matter
    where kernel.py lives or how its host code changes.
    """
    import types

    fixed = os.path.join(BUILD_DIR, "bass_builder_fixed.py")
    try:
        os.makedirs(BUILD_DIR, exist_ok=True)
        old = ""
        if os.path.exists(fixed):
            with open(fixed) as f:
                old = f.read()
        if old != BUILDER_SRC:
            with open(fixed, "w") as f:
                f.write(BUILDER_SRC)
    except Exception:
        pass
    mod = types.ModuleType("bass_builder_fixed")
    code = compile(BUILDER_SRC, fixed, "exec")
    exec(code, mod.__dict__)  # noqa: S102
    return mod


# ---------------------------------------------------------------- host ----

def _quantize(x):
    """Cast x to fp16 and slice into the 8 per-core padded strips (one
    halo row each side)."""
    xh = x.astype(np.float16)
    xpad_all = np.zeros((N_CORES * DIM, ROWS + 2, W + 2), np.float16)
    xp4 = xpad_all.reshape(N_CORES, DIM, ROWS + 2, W + 2)
    for core in range(N_CORES):
        b, quad = divmod(core, 4)
        r0 = quad * ROWS
        j0 = 1 if r0 == 0 else 0
        j1 = ROWS + 1 if r0 + ROWS == H else ROWS + 2
        g0 = r0 - 1 + j0
        xp4[core, :, j0:j1, 1:W + 1] = xh[b, :, g0:g0 + (j1 - j0), :]
    return xpad_all


def _attention_host(grams, proj_w, attca_w, temperature):
    """Per-batch: per-head gram blocks [128, qq|qk|kk] -> attention math
    -> M = proj @ blockdiag(attn)."""
    f = np.float32
    attca2 = attca_w[:, :, 0, 0].astype(f)   # [32, 16]
    projT = proj_w[:, :, 0, 0].T.astype(f)   # [i, o]
    temp128 = np.repeat(temperature.reshape(HEADS).astype(f), C)[:, None]
    ridx = np.arange(DIM)

    ms = []
    for b in range(B):
        g = np.sum([grams[c] for c in GROUPS[b]], axis=0, dtype=np.float64)
        g = g.astype(np.float64)
        sq2 = g[ridx, ridx % C]              # qq block diagonals
        sk2 = g[ridx, 2 * C + ridx % C]      # kk block diagonals
        rq = 1.0 / np.maximum(np.sqrt(sq2), 1e-12)
        rk = 1.0 / np.maximum(np.sqrt(sk2), 1e-12)
        attnb0 = g[:, C:2 * C]               # qk per-head blocks
        rkb = rk.reshape(HEADS, C)[ridx // C, :]
        attn = attnb0 * rkb * rq[:, None] * temp128
        m = attn.max(axis=1, keepdims=True)
        e = np.exp(attn - m)
        attn0 = e / e.sum(axis=1, keepdims=True)
        rl = np.maximum(attn, 0.0)
        r2 = rl * rl
        erf = np.vectorize(math.erf)
        gl = 0.5 * r2 * (1.0 + erf(r2 / math.sqrt(2.0)))
        a1 = gl * r2
        ss = a1 @ attca2.T  # [128, 32]
        attnf = attn0 * (1.0 + ss[:, :C]) + ss[:, C:]
        bd = np.zeros((DIM, DIM))
        for h in range(HEADS):
            bd[h * C:(h + 1) * C, h * C:(h + 1) * C] = attnf[h * C:(h + 1) * C]
        ms.append(np.ascontiguousarray((bd.T @ projT).T.astype(f)))  # M [o, d]
    return ms


# ---------------------------------------------------------------- device ----

def _make_runner(nc, n_cores):
    """Jitted shard_map over 8 cores around the prebuilt bass module --
    the same _bass_exec path run_bass_kernel_spmd uses under axon, minus
    its host-side concat / zero-upload / bulk-result round trips."""
    import jax
    import concourse.bass2jax as bass2jax
    import concourse.mybir as mybir
    from jax.sharding import Mesh, NamedSharding, PartitionSpec
    from jax.experimental.shard_map import shard_map

    bass2jax.install_neuronx_cc_hook()
    assert nc.dbg_addr is None, "debug builds not supported by this runner"
    partition_name = (nc.partition_id_tensor.name
                      if nc.partition_id_tensor else None)

    in_names, out_names, out_avals = [], [], []
    for alloc in nc.m.functions[0].allocations:
        if not isinstance(alloc, mybir.MemoryLocationSet):
            continue
        name = alloc.memorylocations[0].name
        if alloc.kind == "ExternalInput":
            if name != partition_name:
                in_names.append(name)
        elif alloc.kind == "ExternalOutput":
            out_names.append(name)
            out_avals.append(jax.core.ShapedArray(
                tuple(alloc.tensor_shape), mybir.dt.np(alloc.dtype)))
    assert in_names == ["xpad", "qkvt", "dwt"], in_names
    assert out_names == ["pout"], out_names
    n_params, n_outs = len(in_names), len(out_names)
    all_names = in_names + out_names
    if partition_name is not None:
        all_names = all_names + [partition_name]

    def _body(*args):
        operands = list(args)
        if partition_name is not None:
            operands.append(bass2jax.partition_id_tensor())
        outs = bass2jax._bass_exec_p.bind(
            *operands,
            out_avals=tuple(out_avals),
            in_names=tuple(all_names),
            out_names=tuple(out_names),
            lowering_input_output_aliases=(),
            sim_require_finite=True,
            sim_require_nnan=True,
            nc=nc,
        )
        return tuple(outs)

    devices = jax.devices()[:n_cores]
    mesh = Mesh(np.asarray(devices), ("core",))
    in_specs = (PartitionSpec("core"),) * (n_params + n_outs)
    out_specs = (PartitionSpec("core"),) * n_outs
    donate = tuple(range(n_params, n_params + n_outs))
    sharded = jax.jit(
        shard_map(_body, mesh=mesh, in_specs=in_specs, out_specs=out_specs,
                  check_rep=False),
        donate_argnums=donate, keep_unused=True)
    sh = NamedSharding(mesh, PartitionSpec("core"))
    return sharded, sh


def _warmup_into(state):
    """Backend connect + bass build + jit/NEFF compile + executable load,
    run concurrently with host prep.  Ends with device-side zero buffers
    staged for the real call -- no tunnel traffic for any of it."""
    try:
        import jax

        try:
            os.makedirs(CACHE_DIR, exist_ok=True)
            os.environ.setdefault("JAX_COMPILATION_CACHE_DIR", CACHE_DIR)
            jax.config.update("jax_compilation_cache_dir", CACHE_DIR)
            jax.config.update("jax_persistent_cache_min_entry_size_bytes", -1)
            jax.config.update("jax_persistent_cache_min_compile_time_secs", 0.0)
        except Exception:
            pass  # cache is an optimization; run without it if unavailable

        import concourse.bass as bass
        import concourse.bacc as bacc
        import concourse.mybir as mybir
        import concourse.tile as tile
        import jax.numpy as jnp

        bmod = _builder_module()
        holder = {}
        th = threading.Thread(target=bmod.build_kernels_into,
                              name="bass-build",
                              args=(holder, bass, bacc, mybir, tile))
        th.start()
        # Connect + first-touch every device while the bass build runs, so
        # per-process transfer-path warmup stalls land here.
        devs = jax.devices()[:N_CORES]
        futs = [jax.device_put(np.ones((8, 8), np.float32), d) for d in devs]
        for fut in futs:
            fut.block_until_ready()
        state["dev_ready"].set()
        th.join()
        if "err" in holder:
            raise holder["err"]
        state["ncs"] = holder["ncs"]
        (nc_a,) = holder["ncs"]
        sharded, sh = _make_runner(nc_a, N_CORES)
        state["sharded"] = sharded

        # Device-side zeros: dummy inputs for the warmup execution (which
        # triggers the jit/NEFF compile or cache hit plus executable load on
        # all 8 cores) and a fresh donated output buffer for the real call.
        zin = jax.jit(
            lambda: (jnp.zeros((N_CORES * DIM, ROWS + 2, W + 2), jnp.float16),
                     jnp.zeros((N_CORES * DIM, 3 * DIM), jnp.float32),
                     jnp.zeros((N_CORES * 9, 3 * DIM), jnp.float32),
                     jnp.zeros((N_CORES * DIM, PACK_COLS), jnp.uint8)),
            out_shardings=(sh, sh, sh, sh))
        dummies = zin()
        zreal = zin()[3]
        douts = sharded(*dummies)
        jax.block_until_ready(douts)
        state["zeros"] = zreal
        # Warm the downstream path as the LAST warmup step: the first big
        # device->host transfer of a process runs ~80 ms slower than steady
        # state, and the TCP window decays again after idle gaps -- so the
        # warm fetch must end directly before the real call.
        np.asarray(douts[0])
    except BaseException as exc:  # noqa: BLE001
        state["err"] = exc
    finally:
        state["dev_ready"].set()
        state["warm_done"].set()


def _upload_into(state, qkv2T, dwT):
    """Stage inputs to the cores (small replicated weights as soon as the
    backend is up, the bulk int8 x as soon as quantization finishes), then
    issue the device call so execution overlaps the remaining host work."""
    try:
        state["dev_ready"].wait()
        if "err" in state:
            return
        import jax
        from jax.sharding import Mesh, NamedSharding, PartitionSpec

        devs = jax.devices()[:N_CORES]
        sh = NamedSharding(Mesh(np.asarray(devs), ("core",)),
                           PartitionSpec("core"))
        state["qkvt_arr"] = jax.device_put(
            np.tile(np.ascontiguousarray(qkv2T), (N_CORES, 1)), sh)
        state["dwt_arr"] = jax.device_put(
            np.tile(np.ascontiguousarray(dwT), (N_CORES, 1)), sh)
        state["quant_ready"].wait()
        state["x_arr"] = jax.device_put(state["xpad_all"], sh)
        state["warm_done"].wait()
        if "err" in state:
            return
        state["outs"] = state["sharded"](state["x_arr"],
                                         state["qkvt_arr"], state["dwt_arr"],
                                         state["zeros"])
        # Register the device->host copy now so the transfer starts the
        # moment execution completes, overlapping the exec round trip.
        state["outs"][0].copy_to_host_async()
    except BaseException as exc:  # noqa: BLE001
        state["err_upload"] = exc


def _run_fallback(state, xpad_all, qkv2T, dwT):
    """Classic run_bass_kernel_spmd invocation (slower transfers, same
    kernel) -- used only if the custom runner path fails."""
    from concourse import bass_utils

    (nc_a,) = state["ncs"]
    xp4 = xpad_all.reshape(N_CORES, DIM, ROWS + 2, W + 2)
    in_maps = [{"xpad": xp4[c], "qkvt": qkv2T, "dwt": dwT}
               for c in range(N_CORES)]
    res = bass_utils.run_bass_kernel_spmd(nc_a, in_maps, list(range(N_CORES)))
    return np.stack([r["pout"] for r in res.results])


# ------------------------------------------------------------- emulation ----

def _emulate_device(xpad_all, qkv2T, dwT):
    """Numpy re-implementation of the device kernel: packed outputs."""
    f = np.float32
    wqk_f = np.empty((DIM, 9 * 2 * DIM), dtype=f)
    wv_f = np.empty((DIM, 9 * DIM), dtype=f)
    for tap in range(9):
        prod = qkv2T * dwT[tap][None, :]
        wqk_f[:, tap * 2 * DIM:(tap + 1) * 2 * DIM] = \
            prod[:, :2 * DIM].astype(np.float16).astype(f)
        wv_f[:, tap * DIM:(tap + 1) * DIM] = \
            prod[:, 2 * DIM:].astype(np.float16).astype(f)
    xp4 = xpad_all.reshape(N_CORES, DIM, ROWS + 2, W + 2)
    packed = np.zeros((N_CORES, DIM, PACK_COLS), np.uint8)
    for core in range(N_CORES):
        xp = xp4[core].astype(f)
        qk = np.zeros((2 * DIM, ROWS, W), dtype=f)
        v = np.zeros((DIM, ROWS, W), dtype=f)
        for tap in range(9):
            dh, dw = divmod(tap, 3)
            xs = xp[:, dh:dh + ROWS, dw:dw + W]
            qk += np.einsum('io,ihw->ohw',
                            wqk_f[:, tap * 2 * DIM:(tap + 1) * 2 * DIM], xs)
            v += np.einsum('io,ihw->ohw',
                           wv_f[:, tap * DIM:(tap + 1) * DIM], xs)
        q = qk[:DIM].reshape(DIM, L_CORE)
        k = qk[DIM:].reshape(DIM, L_CORE)
        g = np.empty((DIM, 3 * DIM), dtype=f)
        g[:, :DIM] = q @ q.T
        g[:, DIM:2 * DIM] = q @ k.T
        g[:, 2 * DIM:] = k @ k.T
        vt = v.reshape(DIM, NTILE, 512)
        m = np.maximum(np.abs(vt).max(axis=2, keepdims=True), 1e-30)
        u = (np.rint(vt * (63.0 / m)) + 64.0).astype(np.uint8)
        ug = u.reshape(DIM, NTILE, 64, 8)
        pk = (ug[..., :7] & 0x7F) | \
            (((ug[..., 7:8] >> np.arange(7, dtype=np.uint8)) & 1) << 7)
        packed[core, :, :PACK_V] = pk.reshape(DIM, PACK_V)
        gc = np.empty((DIM, 3 * C), dtype=f)
        for h in range(HEADS):
            p0 = h * C
            gc[p0:p0 + C, 0:C] = g[p0:p0 + C, p0:p0 + C]
            gc[p0:p0 + C, C:2 * C] = g[p0:p0 + C, DIM + p0:DIM + p0 + C]
            gc[p0:p0 + C, 2 * C:] = g[p0:p0 + C, 2 * DIM + p0:2 * DIM + p0 + C]
        packed[core, :, PACK_V:PACK_V + PACK_G] = gc.view(np.uint8)
        packed[core, :, PACK_V + PACK_G:] = \
            np.ascontiguousarray((m[:, :, 0] / 63.0).astype(np.float16)
                                 ).view(np.uint8)
    return packed


# ---------------------------------------------------------------- entry ----

def kernel(x, qkv_w, dw_w, proj_w, attca_w, temperature):
    t_start = time.perf_counter()
    x = np.ascontiguousarray(np.asarray(x, dtype=np.float32))
    qkv_w = np.asarray(qkv_w, dtype=np.float32)
    dw_w = np.asarray(dw_w, dtype=np.float32)
    proj_w = np.asarray(proj_w, dtype=np.float32)
    attca_w = np.asarray(attca_w, dtype=np.float32)
    temperature = np.asarray(temperature, dtype=np.float32)

    qkv2T = np.ascontiguousarray(qkv_w[:, :, 0, 0].T)            # [128, 384]
    dwT = np.ascontiguousarray(dw_w[:, 0].reshape(3 * DIM, 9).T)  # [9, 384]

    emulate = os.environ.get("KERNEL_EMULATE", "0") == "1"
    state = {
        "dev_ready": threading.Event(),
        "quant_ready": threading.Event(),
        "warm_done": threading.Event(),
    }
    if not emulate:
        wth = threading.Thread(target=_warmup_into, args=(state,), daemon=True)
        uth = threading.Thread(target=_upload_into, args=(state, qkv2T, dwT),
                               daemon=True)
        wth.start()
        uth.start()

    xpad_all = _quantize(x)
    state["xpad_all"] = xpad_all
    state["quant_ready"].set()

    if emulate:
        packed = _emulate_device(xpad_all, qkv2T, dwT)
        t1 = t2 = time.perf_counter()
    else:
        wth.join()
        uth.join()
        t1 = time.perf_counter()
        try:
            if "err" in state:
                raise state["err"]
            if "err_upload" in state:
                raise state["err_upload"]
            packed = np.asarray(state["outs"][0]).reshape(
                N_CORES, DIM, PACK_COLS)
        except Exception:
            try:
                if "ncs" not in state:
                    raise
                packed = _run_fallback(state, xpad_all, qkv2T, dwT)
            except Exception:
                packed = _emulate_device(xpad_all, qkv2T, dwT)
        t2 = time.perf_counter()

    grams = [np.ascontiguousarray(packed[c, :, PACK_V:PACK_V + PACK_G]
                                  ).view(np.float32)
             for c in range(N_CORES)]
    ms = _attention_host(grams, proj_w, attca_w, temperature)
    out = np.empty((B, DIM, H, W), dtype=np.float32)
    for core in range(N_CORES):
        b, quad = divmod(core, 4)
        r0 = quad * ROWS
        vs = np.ascontiguousarray(packed[core, :, PACK_V + PACK_G:]
                                  ).view(np.float16).astype(np.float32)
        pk = packed[core, :, :PACK_V].reshape(DIM, NTILE, 64, 7)
        u = np.empty((DIM, NTILE, 64, 8), np.uint8)
        u[..., :7] = pk & 0x7F
        u[..., 7] = ((pk >> 7).astype(np.uint16) <<
                     np.arange(7, dtype=np.uint16)).sum(
                         axis=-1, dtype=np.uint16).astype(np.uint8)
        vq = u.reshape(DIM, NTILE, 512).astype(np.float32) - 64.0
        v = (vq * vs[:, :, None]).reshape(DIM, L_CORE)
        out[b, :, r0:r0 + ROWS, :] = (ms[core // 4] @ v).reshape(DIM, ROWS, W)
    t3 = time.perf_counter()

    LAST_TIMING["build_a_s"] = t1 - t_start
    LAST_TIMING["run_a_s"] = t2 - t1
    LAST_TIMING["host_s"] = t3 - t2
    LAST_TIMING["run_b_s"] = 0.0
    LAST_TIMING["kernel_a_ns"] = None
    LAST_TIMING["kernel_b_ns"] = None
    return out


# Preload heavyweight modules at import time so kernel() only pays for the
# device connect, compiles and transfers.
try:
    import jax  # noqa: F401,E402
    import concourse.bass  # noqa: F401,E402
    import concourse.bacc  # noqa: F401,E402
    import concourse.mybir  # noqa: F401,E402
    import concourse.tile  # noqa: F401,E402
    import concourse.bass2jax  # noqa: F401,E402
    from concourse import bass_utils  # noqa: F401,E402
except Exception:  # pragma: no cover - emulation-only environments
    pass
